# revision 15
# baseline (speedup 1.0000x reference)
"""Trainium2 Bass kernel for CapsuleLayer (nn_CapsuleLayer_45552423142009).

Computes, for x[B,768]:
  u = squash(x @ Wp + bp)            # [B, 8, 16]  (squash over last dim)
  u_hat[b,p,c,:] = u[b,p,:] @ W[p,c] # [B, 8, 5, 16]
  3 iterations of dynamic routing -> v [B, 5, 16]

Strategy: pure data-parallel over 8 NeuronCores (batch sharded 16384/core).
On-chip layout is feature-major: features on partitions, batch on the free
dim (512-wide tiles).  All matmul traffic is bf16 (fp32 PSUM accumulate);
u_hat is never materialized - the agreement a[p,c] = uh . v is computed as
u . (W_c v_c) with two small matmuls per class.  The per-sample scalar
chains (squash factors, softmax) are packed across the two tiles of a pair
so one ACT op serves both tiles.  Elementwise muls with PSUM operands run
on DVE; SBUF-only squares and stt ops run on GPSIMD (Pool); casts/exp/ln on
ACT.  Tiles are processed in software-pipelined pairs; per-class emission
is pipelined (bcast matmul -> DVE mul -> consumer matmul with a 1-2 class
lag) and the next pair's PE work is injected at the serial joins so the PE
never idles (keeps the p-state ramp at full clock).
"""

import sys
import numpy as np

sys.path.insert(0, "/opt/trn_rl_repo")

from concourse import bass, bacc, mybir  # noqa: E402
from concourse import tile  # noqa: E402
from concourse.bass_utils import run_bass_kernel_spmd  # noqa: E402
from concourse.alu_op_type import AluOpType  # noqa: E402

import ml_dtypes  # noqa: E402

F32 = mybir.dt.float32
BF16 = mybir.dt.bfloat16
AF = mybir.ActivationFunctionType

B = 131072
D = 768
P = 8
PD = 16
C = 5
CD = 16
NCORES = 8
BC = B // NCORES          # 16384 batch rows per core
NB = 512                  # batch columns per tile
NT = BC // NB             # 32 tiles

# const blob column offsets (bf16, [128, CST_W]).  Selectors consumed as
# lhsT against a row-offset rhs are duplicated at matching row offsets
# (PE tile rule: lhsT and rhs must share their base partition).
S_SSEL8 = 0                      # [128, 8]   sum (p,i)-groups -> p
S_SBC = 8                        # [8, 128]   broadcast p -> (p,i)  (+ rows 32-39)
S_CSEL = 136                     # [40, 8]    sum over c            (+ rows 64-103)
S_CBC = 144                      # [8, 40]    broadcast p -> (c,p)  (+ rows 32-39)
S_BSEL = 184                     # [40, 640]  5 x [40,128]: (c,p) -> (p,i)  (+ rows 64-103)
S_JSEL = 824                     # [80, 5]    sum over j at fixed c
S_JBC = 829                      # [5, 80]    broadcast c -> (c,j)  (+ rows 32-36)
S_ARED = 909                     # [128, 200] 5 x [128,40]: sum_i -> (c,p)
O_WFM = 1109                     # [128, 400] 5 x class-masked wflat
O_WF02 = 1509                    # [128, 80]  0.2 * wflat
O_WV = 1589                     # [80, 640]  5 x [80,128] masked: Wv_c[(c',j),(p,i)]
O_WP = 2229                      # [128, 768] mm1 weights
O_BP = 2997                      # [128, 1]   bias
CST_W = 2998


class _BaccOneActTable(bacc.Bacc):
    """Pin every activation to the natural_log_exp table (no table thrash)."""

    _TABLE = "natural_log_exp_and_others"

    def insert_act_table_loads(self):
        import bass_rust as _bass_rust
        from concourse.hw_specs import get_activation_tables

        has_activation = any(
            isinstance(i, mybir.InstActivation)
            for b in self.main_func.blocks
            for i in b.instructions
        )
        if not has_activation:
            return
        tables = [
            (name, funcs if name == self._TABLE else set())
            for name, funcs in get_activation_tables(self.m.arch).items()
        ]
        _bass_rust.insert_act_table_loads(self, tables)


def build_selectors() -> np.ndarray:
    sel = np.zeros((128, CST_W), dtype=np.float32)
    for p in range(P):
        for i in range(PD):
            sel[p * 16 + i, S_SSEL8 + p] = 1.0                 # Ssel8
            sel[p, S_SBC + p * 16 + i] = 1.0                   # Sbc
            sel[32 + p, S_SBC + p * 16 + i] = 1.0              # Sbc copy
    for t, r0 in ((0, 0), (1, 64)):
        for c in range(C):
            for p in range(P):
                sel[r0 + c * 8 + p, S_CSEL + p] = 1.0          # Csel (+copy)
    for t, r0 in ((0, 0), (1, 32)):
        for c in range(C):
            for p in range(P):
                sel[r0 + p, S_CBC + c * 8 + p] = 1.0           # Cbc (+copy)
    for r0 in (0, 64):
        for c in range(C):
            for p in range(P):
                sel[r0 + c * 8 + p, S_BSEL + c * 128 + p * 16:
                    S_BSEL + c * 128 + (p + 1) * 16] = 1.0     # Bsel_c (+copy)
    for c in range(C):
        for j in range(CD):
            sel[c * 16 + j, S_JSEL + c] = 1.0                  # Jsel
    for r0 in (0, 32):
        for c in range(C):
            for j in range(CD):
                sel[r0 + c, S_JBC + c * 16 + j] = 1.0          # Jbc (+copy)
    for c in range(C):
        for p in range(P):
            for i in range(PD):
                # Ared_c: [(p,i), (c',p')] = d_c'c d_p'p
                sel[p * 16 + i, S_ARED + c * 40 + c * 8 + p] = 1.0
    return sel


def build_nc(nt: int = NT) -> bass.Bass:
    assert nt % 2 == 0
    npairs = nt // 2
    bc = nt * NB
    nc = _BaccOneActTable(None)

    x_d = nc.declare_dram_parameter("xt", [D, bc], BF16, isOutput=False)
    cst_d = nc.declare_dram_parameter("cst", [128, CST_W], BF16, isOutput=False)
    v_d = nc.declare_dram_parameter("vout", [C * CD, bc], F32, isOutput=True)

    with tile.TileContext(nc) as tc, nc.allow_low_precision(reason="bf16 matmul/elementwise"):
        with (
            tc.sbuf_pool(name="const", bufs=1) as cpool,
            tc.sbuf_pool(name="xt", bufs=8) as xtpool,
            tc.sbuf_pool(name="mid", bufs=5) as mpool,
            tc.sbuf_pool(name="rt", bufs=2) as rtpool,
            tc.sbuf_pool(name="sm", bufs=4) as smpool,
            tc.psum_pool(name="pmm", bufs=2) as pmmp,
            tc.psum_pool(name="pbc", bufs=3) as pbcp,
            tc.psum_pool(name="psc", bufs=3) as pscp,
        ):
            # ---- constants: one DMA, staged through DVE so consumers depend
            # on the DVE semaphore ----
            cst0 = cpool.tile([128, CST_W], BF16)
            nc.sync.dma_start(out=cst0[:], in_=cst_d[:])
            cst = cpool.tile([128, CST_W], BF16)
            nc.vector.tensor_copy(cst[:], cst0[:])
            ssel8 = cst[:, S_SSEL8:S_SSEL8 + 8]

            def sbc8(t):
                return cst[32 * t:32 * t + 8, S_SBC:S_SBC + 128]

            def csel(t):
                return cst[64 * t:64 * t + 40, S_CSEL:S_CSEL + 8]

            def cbc(t):
                return cst[32 * t:32 * t + 8, S_CBC:S_CBC + 40]

            jsel = cst[:80, S_JSEL:S_JSEL + 5]

            def jbc(t):
                return cst[32 * t:32 * t + 5, S_JBC:S_JBC + 80]
            wp_sb = cst[:, O_WP:O_WP + 768]
            wfm_sb = cst[:, O_WFM:O_WFM + 400]
            wf02_sb = cst[:, O_WF02:O_WF02 + 80]
            bp_sb = cst[:, O_BP:O_BP + 1]

            def bsel_c(c, t):
                return cst[64 * t:64 * t + 40,
                           S_BSEL + c * 128:S_BSEL + (c + 1) * 128]

            def ared_c(c):
                return cst[:, S_ARED + c * 40:S_ARED + (c + 1) * 40]

            def wv_c(c):
                return cst[:80, O_WV + c * 128:O_WV + (c + 1) * 128]

            class TS:
                """Per-tile state."""
                def __init__(self, it):
                    self.it = it

            class PairS:
                def __init__(self, k):
                    self.k = k
                    self.A = TS(2 * k)
                    self.B = TS(2 * k + 1)
                    self.ts = (self.A, self.B)

            def noop():
                pass

            # ---------------- phase H: load + mm1 + squash-u ----------------
            def h_dma(pr):
                for s in pr.ts:
                    s.xT = xtpool.tile([128, 6, NB], BF16, tag="xt")
                    src = x_d[:, s.it * NB:(s.it + 1) * NB].rearrange(
                        "(k p) b -> p k b", p=128)
                    nc.sync.dma_start(out=s.xT[:], in_=src)

            def h_mm1(pr, which):
                s = pr.ts[which]
                s.pu = pmmp.tile([128, NB], F32, tag="pmm")
                for k in range(6):
                    nc.tensor.matmul(
                        s.pu[:], wp_sb[:, k * 128:(k + 1) * 128],
                        s.xT[:, k, :], start=(k == 0), stop=(k == 5))

            def h_upre(pr, which):
                s = pr.ts[which]
                s.u_pre = mpool.tile([128, NB], BF16, tag="mid")
                nc.scalar.activation(s.u_pre[:], s.pu[:], AF.Identity,
                                     bias=bp_sb, scale=1.0)
                s.usq = mpool.tile([128, NB], BF16, tag="mid2")
                nc.scalar.activation(s.usq[:], s.pu[:], AF.Square,
                                     bias=bp_sb, scale=1.0)

            def h_psq(pr):
                # packed sq: tile A -> rows 0-7, tile B -> rows 32-39 of one
                # psum column; the gap rows hold garbage that is never read.
                psq = pbcp.tile([40, NB], F32, tag="pbc", name="psq")
                for t, s in enumerate(pr.ts):
                    nc.tensor.matmul(
                        psq[32 * t:32 * t + 8, :], ssel8,
                        s.usq[:], start=True, stop=True)
                w = smpool.tile([40, NB], F32, tag="sm")
                nc.scalar.activation(w[:], psq[:], AF.Ln)
                l1 = smpool.tile([40, NB], F32, tag="sm")
                nc.scalar.activation(l1[:], psq[:], AF.Ln,
                                     bias=1.0, scale=1.0)
                zf = smpool.tile([40, NB], F32, tag="sm")
                nc.vector.scalar_tensor_tensor(
                    zf[:], w[:], 0.5, l1[:],
                    op0=AluOpType.mult, op1=AluOpType.subtract)
                pr.fz = smpool.tile([40, NB], BF16, tag="smb")
                nc.scalar.activation(pr.fz[:], zf[:], AF.Exp)

            def h_u(pr):
                for t, s in enumerate(pr.ts):
                    pfb = pbcp.tile([128, NB], F32, tag="pbc")
                    nc.tensor.matmul(pfb[:], sbc8(t),
                                     pr.fz[32 * t:32 * t + 8, :],
                                     start=True, stop=True)
                    s.u = mpool.tile([128, NB], BF16, tag="mid3")
                    nc.vector.tensor_mul(s.u[:], s.u_pre[:], pfb[:])

            # ---------------- routing pieces ----------------
            def r_smm0(pr):
                for s in pr.ts:
                    s.psc = pscp.tile([80, NB], F32, tag="psc")
                    nc.tensor.matmul(s.psc[:], wf02_sb, s.u[:],
                                     start=True, stop=True)

            def r_soft(pr):
                # softmax on packed logits [104, NB] (tile B at row 64)
                pr.e = rtpool.tile([104, NB], BF16, tag="rt_e")
                nc.scalar.activation(pr.e[:], pr.logits[:], AF.Exp)
                pden = pbcp.tile([40, NB], F32, tag="pbc", name="pden")
                for t in range(2):
                    nc.tensor.matmul(pden[32 * t:32 * t + 8, :], csel(t),
                                     pr.e[64 * t:64 * t + 40, :],
                                     start=True, stop=True)
                rdr32 = smpool.tile([40, NB], F32, tag="sm")
                nc.vector.reciprocal_approx_fast(out=rdr32[:], in_=pden[:])
                rdr = smpool.tile([40, NB], BF16, tag="smb2")
                nc.scalar.copy(rdr[:], rdr32[:])
                pdb = pbcp.tile([104, NB], F32, tag="pbc", name="pdb")
                for t in range(2):
                    nc.tensor.matmul(pdb[64 * t:64 * t + 40, :], cbc(t),
                                     rdr[32 * t:32 * t + 8, :],
                                     start=True, stop=True)
                pr.cn = rtpool.tile([104, NB], BF16, tag="rt_cn")
                nc.vector.tensor_mul(pr.cn[:], pr.e[:], pdb[:])

            def r_s(prs):
                # per-class pipelined across the group:
                # [4x bcast mm] [4x ts mul] [4x wfm(c-1)]
                for pr in prs:
                    for s in pr.ts:
                        s.tsb = rtpool.tile([128, C, NB], BF16,
                                            tag="rt_ts", name="tsb",
                                            bufs=4)
                        s.pcb = [None] * C
                        s.psc = pscp.tile([80, NB], F32, tag="psc",
                                          name="psc")
                for c in range(C):
                    for pr in prs:
                        for t, s in enumerate(pr.ts):
                            pcb = pbcp.tile([128, NB], F32, tag="pbc",
                                            name="pcb")
                            nc.tensor.matmul(
                                pcb[:], bsel_c(c, t),
                                pr.cn[64 * t:64 * t + 40, :],
                                start=True, stop=True)
                            s.pcb[c] = pcb
                        for s in pr.ts:
                            nc.vector.tensor_mul(
                                s.tsb[:, c, :], s.u[:], s.pcb[c][:])
                    if c >= 2:
                        for pr in prs:
                            for s in pr.ts:
                                nc.tensor.matmul(
                                    s.psc[:],
                                    wfm_sb[:, (c - 2) * 80:(c - 1) * 80],
                                    s.tsb[:, c - 2, :],
                                    start=(c == 2), stop=False)
                for c in (3, 4):
                    for pr in prs:
                        for s in pr.ts:
                            nc.tensor.matmul(
                                s.psc[:], wfm_sb[:, c * 80:(c + 1) * 80],
                                s.tsb[:, c, :], start=False, stop=(c == 4))

            def r_v_head(pr):
                # s80 cast + ssq + jsel into packed vsq (rows 0-4 / 32-36)
                pvq = pbcp.tile([37, NB], F32, tag="pbc", name="pvq")
                pr.pvq = pvq
                for t, s in enumerate(pr.ts):
                    s.s80 = rtpool.tile([80, NB], BF16, tag=f"rt_s{t}",
                                        name="s80")
                    nc.scalar.copy(s.s80[:], s.psc[:])
                for t, s in enumerate(pr.ts):
                    s.ssq = rtpool.tile([80, NB], BF16, tag=f"rt_q{t}",
                                        name="ssq")
                    nc.scalar.activation(s.ssq[:], s.psc[:], AF.Square)
                for t, s in enumerate(pr.ts):
                    nc.tensor.matmul(pvq[32 * t:32 * t + 5, :],
                                     jsel, s.ssq[:], start=True, stop=True)

            def r_g(pr):
                pvq = pr.pvq
                wg = smpool.tile([37, NB], F32, tag="sm")
                nc.scalar.activation(wg[:], pvq[:], AF.Ln)
                lg1 = smpool.tile([37, NB], F32, tag="sm")
                nc.scalar.activation(lg1[:], pvq[:], AF.Ln,
                                     bias=1.0, scale=1.0)
                zg = smpool.tile([37, NB], F32, tag="sm")
                nc.vector.scalar_tensor_tensor(
                    zg[:], wg[:], 0.5, lg1[:],
                    op0=AluOpType.mult, op1=AluOpType.subtract)
                pr.g = smpool.tile([37, NB], BF16, tag="smb")
                nc.scalar.activation(pr.g[:], zg[:], AF.Exp)

            def r_v(pr, itr):
                for t, s in enumerate(pr.ts):
                    pgb = pbcp.tile([80, NB], F32, tag="pbc", name="pgb")
                    nc.tensor.matmul(pgb[:], jbc(t),
                                     pr.g[32 * t:32 * t + 5, :],
                                     start=True, stop=True)
                    dt = F32 if itr == 2 else BF16
                    s.v = rtpool.tile([80, NB], dt, tag=f"rt_v{t}", name="v")
                    nc.vector.tensor_mul(s.v[:], s.s80[:], pgb[:])
                    if itr == 2:
                        nc.sync.dma_start(
                            out=v_d[:, s.it * NB:(s.it + 1) * NB], in_=s.v[:])

            def r_ag(prs, itr):
                for pr in prs:
                    for s in pr.ts:
                        s.au = rtpool.tile([128, C, NB], BF16,
                                           tag="rt_au", name="au", bufs=4)
                        s.m = [None] * C
                    pr.pat = pscp.tile([104, NB], F32, tag="psc",
                                       name="pat")
                for c in range(C):
                    for pr in prs:
                        for t, s in enumerate(pr.ts):
                            m = pmmp.tile([128, NB], F32, tag="pmm",
                                          name="m")
                            nc.tensor.matmul(m[:], wv_c(c), s.v[:],
                                             start=True, stop=True)
                            s.m[c] = m
                        for s in pr.ts:
                            nc.vector.tensor_mul(
                                s.au[:, c, :], s.u[:], s.m[c][:])
                    if c >= 2:
                        for pr in prs:
                            for t, s in enumerate(pr.ts):
                                nc.tensor.matmul(
                                    pr.pat[64 * t:64 * t + 40, :],
                                    ared_c(c - 2), s.au[:, c - 2, :],
                                    start=(c == 2), stop=False)
                for c in (3, 4):
                    for pr in prs:
                        for t, s in enumerate(pr.ts):
                            nc.tensor.matmul(
                                pr.pat[64 * t:64 * t + 40, :], ared_c(c),
                                s.au[:, c, :], start=False, stop=(c == 4))

            def r_logit(pr, itr):
                if itr == 0:
                    pr.logits = rtpool.tile([104, NB], F32, tag="rt_lg",
                                            name="logits")
                    nc.scalar.copy(pr.logits[:], pr.pat[:])
                else:
                    lg2 = rtpool.tile([104, NB], F32, tag="rt_lg2")
                    nc.vector.tensor_add(lg2[:], pr.logits[:], pr.pat[:])
                    pr.logits = lg2

            # ---------------- composed group phases ----------------
            def r0(prs):
                for pr in prs:
                    r_smm0(pr)
                for pr in prs:
                    r_v_head(pr)
                for pr in prs:
                    r_g(pr)
                for pr in prs:
                    r_v(pr, 0)
                r_ag(prs, 0)
                for pr in prs:
                    r_logit(pr, 0)

            def r12(prs, itr):
                for pr in prs:
                    r_soft(pr)
                r_s(prs)
                for pr in prs:
                    r_v_head(pr)
                for pr in prs:
                    r_g(pr)
                for pr in prs:
                    r_v(pr, itr)
                if itr == 1:
                    r_ag(prs, 1)
                    for pr in prs:
                        r_logit(pr, 1)

            pairs = [PairS(k) for k in range(npairs)]

            def h_all(prs):
                for pr in prs:
                    h_mm1(pr, 0)
                    h_mm1(pr, 1)
                for pr in prs:
                    h_upre(pr, 0)
                    h_upre(pr, 1)
                for pr in prs:
                    h_psq(pr)
                for pr in prs:
                    h_u(pr)

            assert npairs % 2 == 0
            groups = [pairs[2 * g:2 * g + 2] for g in range(npairs // 2)]
            for pr in groups[0]:
                h_dma(pr)
            for gi, grp in enumerate(groups):
                h_all(grp)
                if gi + 1 < len(groups):
                    for pr in groups[gi + 1]:
                        h_dma(pr)
                r0(grp)
                r12(grp, 1)
                r12(grp, 2)

    nc.compile()
    return nc


_NC_CACHE: dict = {}


def _get_nc(nt: int) -> bass.Bass:
    if nt not in _NC_CACHE:
        _NC_CACHE[nt] = build_nc(nt)
    return _NC_CACHE[nt]


def _prep_weights(Wp, bp, W):
    Wp = np.asarray(Wp, np.float32)
    bp = np.asarray(bp, np.float32)
    W = np.asarray(W, np.float32)
    wp_flat = Wp.transpose(1, 0, 2).reshape(768, 128)          # [d, (p,i)]
    wp_h = np.ascontiguousarray(
        wp_flat.reshape(6, 128, 128).transpose(1, 0, 2).reshape(128, 768))
    wflat_h = np.ascontiguousarray(
        W.transpose(0, 2, 1, 3).reshape(128, C * CD))          # [(p,i), (c,j)]
    wf02_h = np.ascontiguousarray(0.2 * wflat_h)
    wfm_h = np.zeros((128, C, 80), np.float32)
    for c in range(C):
        wfm_h[:, c, c * 16:(c + 1) * 16] = wflat_h[:, c * 16:(c + 1) * 16]
    wfm_h = np.ascontiguousarray(wfm_h.reshape(128, 400))
    # Wv_c[(c',j), (p,i)] = d_c'c W[p,c,i,j]  (class-masked, rhs is full v)
    wv_h = np.zeros((80, C, 128), np.float32)
    for c in range(C):
        wv_h[c * 16:(c + 1) * 16, c, :] = \
            W[:, c, :, :].transpose(2, 0, 1).reshape(16, 128)
    wv_h = np.ascontiguousarray(wv_h.reshape(80, C * 128))
    bp_h = np.ascontiguousarray(bp.reshape(128, 1))
    return wp_h, wflat_h, wf02_h, wfm_h, wv_h, bp_h


def pack_consts(Wp, bp, W):
    wp_h, wflat_h, wf02_h, wfm_h, wv_h, bp_h = _prep_weights(Wp, bp, W)
    cst = build_selectors()
    cst[:, O_WP:O_WP + 768] = wp_h
    cst[:, O_WFM:O_WFM + 400] = wfm_h
    cst[:, O_WF02:O_WF02 + 80] = wf02_h
    cst[:80, O_WV:O_WV + 640] = wv_h
    cst[:, O_BP:O_BP + 1] = bp_h
    return np.ascontiguousarray(cst.astype(ml_dtypes.bfloat16))


def make_in_maps(inputs, nt=NT):
    x = np.asarray(inputs["x"], np.float32)
    cst = pack_consts(inputs["Wp"], inputs["bp"], inputs["W"])
    bc = nt * NB
    return [{"xt": np.ascontiguousarray(
                x[i * BC:i * BC + bc].T.astype(ml_dtypes.bfloat16)),
             "cst": cst}
            for i in range(NCORES)]


def collect_out(res, nt=NT):
    # vout is [(c,j) = 80, bc] feature-major; transpose back to [b, c, j]
    outs = [np.asarray(res.results[i]["vout"]).reshape(C, CD, -1)
            .transpose(2, 0, 1) for i in range(NCORES)]
    return np.ascontiguousarray(np.concatenate(outs, axis=0).astype(np.float32))


def kernel(x, Wp, bp, W):
    nc = _get_nc(NT)
    in_maps = make_in_maps({"x": x, "Wp": Wp, "bp": bp, "W": W})
    res = run_bass_kernel_spmd(nc, in_maps, list(range(NCORES)))
    return collect_out(res)


# revision 16
# speedup vs baseline: 1.0054x; 1.0054x over previous
"""Trainium2 Bass kernel for CapsuleLayer (nn_CapsuleLayer_45552423142009).

Computes, for x[B,768]:
  u = squash(x @ Wp + bp)            # [B, 8, 16]  (squash over last dim)
  u_hat[b,p,c,:] = u[b,p,:] @ W[p,c] # [B, 8, 5, 16]
  3 iterations of dynamic routing -> v [B, 5, 16]

Strategy: pure data-parallel over 8 NeuronCores (batch sharded 16384/core).
On-chip layout is feature-major: features on partitions, batch on the free
dim (512-wide tiles).  All matmul traffic is bf16 (fp32 PSUM accumulate);
u_hat is never materialized - the agreement a[p,c] = uh . v is computed as
u . (W_c v_c) with two small matmuls per class.  The per-sample scalar
chains (squash factors, softmax) are packed across the two tiles of a pair
so one ACT op serves both tiles.  Elementwise muls with PSUM operands run
on DVE; SBUF-only squares and stt ops run on GPSIMD (Pool); casts/exp/ln on
ACT.  Tiles are processed in software-pipelined pairs; per-class emission
is pipelined (bcast matmul -> DVE mul -> consumer matmul with a 1-2 class
lag) and the next pair's PE work is injected at the serial joins so the PE
never idles (keeps the p-state ramp at full clock).
"""

import sys
import numpy as np

sys.path.insert(0, "/opt/trn_rl_repo")

from concourse import bass, bacc, mybir  # noqa: E402
from concourse import tile  # noqa: E402
from concourse.bass_utils import run_bass_kernel_spmd  # noqa: E402
from concourse.alu_op_type import AluOpType  # noqa: E402

import ml_dtypes  # noqa: E402

F32 = mybir.dt.float32
BF16 = mybir.dt.bfloat16
AF = mybir.ActivationFunctionType

B = 131072
D = 768
P = 8
PD = 16
C = 5
CD = 16
NCORES = 8
BC = B // NCORES          # 16384 batch rows per core
NB = 512                  # batch columns per tile
NT = BC // NB             # 32 tiles

# const blob column offsets (bf16, [128, CST_W]).  Selectors consumed as
# lhsT against a row-offset rhs are duplicated at matching row offsets
# (PE tile rule: lhsT and rhs must share their base partition).
S_SSEL8 = 0                      # [128, 8]   sum (p,i)-groups -> p
S_SBC = 8                        # [8, 128]   broadcast p -> (p,i)  (+ rows 32-39)
S_CSEL = 136                     # [40, 8]    sum over c            (+ rows 64-103)
S_CBC = 144                      # [8, 40]    broadcast p -> (c,p)  (+ rows 32-39)
S_BSEL = 184                     # [40, 640]  5 x [40,128]: (c,p) -> (p,i)  (+ rows 64-103)
S_JSEL = 824                     # [80, 5]    sum over j at fixed c
S_JBC = 829                      # [5, 80]    broadcast c -> (c,j)  (+ rows 32-36)
S_ARED = 909                     # [128, 200] 5 x [128,40]: sum_i -> (c,p)
O_WFM = 1109                     # [128, 400] 5 x class-masked wflat
O_WF02 = 1509                    # [128, 80]  0.2 * wflat
O_WV = 1589                     # [80, 640]  5 x [80,128] masked: Wv_c[(c',j),(p,i)]
O_WP = 2229                      # [128, 768] mm1 weights
O_BP = 2997                      # [128, 1]   bias
CST_W = 2998


class _BaccOneActTable(bacc.Bacc):
    """Pin every activation to the natural_log_exp table (no table thrash)."""

    _TABLE = "natural_log_exp_and_others"

    def insert_act_table_loads(self):
        import bass_rust as _bass_rust
        from concourse.hw_specs import get_activation_tables

        has_activation = any(
            isinstance(i, mybir.InstActivation)
            for b in self.main_func.blocks
            for i in b.instructions
        )
        if not has_activation:
            return
        tables = [
            (name, funcs if name == self._TABLE else set())
            for name, funcs in get_activation_tables(self.m.arch).items()
        ]
        _bass_rust.insert_act_table_loads(self, tables)


def build_selectors() -> np.ndarray:
    sel = np.zeros((128, CST_W), dtype=np.float32)
    for p in range(P):
        for i in range(PD):
            sel[p * 16 + i, S_SSEL8 + p] = 1.0                 # Ssel8
            sel[p, S_SBC + p * 16 + i] = 1.0                   # Sbc
            sel[32 + p, S_SBC + p * 16 + i] = 1.0              # Sbc copy
    for t, r0 in ((0, 0), (1, 64)):
        for c in range(C):
            for p in range(P):
                sel[r0 + c * 8 + p, S_CSEL + p] = 1.0          # Csel (+copy)
    for t, r0 in ((0, 0), (1, 32)):
        for c in range(C):
            for p in range(P):
                sel[r0 + p, S_CBC + c * 8 + p] = 1.0           # Cbc (+copy)
    for r0 in (0, 64):
        for c in range(C):
            for p in range(P):
                sel[r0 + c * 8 + p, S_BSEL + c * 128 + p * 16:
                    S_BSEL + c * 128 + (p + 1) * 16] = 1.0     # Bsel_c (+copy)
    for c in range(C):
        for j in range(CD):
            sel[c * 16 + j, S_JSEL + c] = 1.0                  # Jsel
    for r0 in (0, 32):
        for c in range(C):
            for j in range(CD):
                sel[r0 + c, S_JBC + c * 16 + j] = 1.0          # Jbc (+copy)
    for c in range(C):
        for p in range(P):
            for i in range(PD):
                # Ared_c: [(p,i), (c',p')] = d_c'c d_p'p
                sel[p * 16 + i, S_ARED + c * 40 + c * 8 + p] = 1.0
    return sel


def build_nc(nt: int = NT) -> bass.Bass:
    assert nt % 2 == 0
    npairs = nt // 2
    bc = nt * NB
    nc = _BaccOneActTable(None)

    x_d = nc.declare_dram_parameter("xt", [D, bc], BF16, isOutput=False)
    cst_d = nc.declare_dram_parameter("cst", [128, CST_W], BF16, isOutput=False)
    v_d = nc.declare_dram_parameter("vout", [C * CD, bc], F32, isOutput=True)

    with tile.TileContext(nc) as tc, nc.allow_low_precision(reason="bf16 matmul/elementwise"):
        with (
            tc.sbuf_pool(name="const", bufs=1) as cpool,
            tc.sbuf_pool(name="xt", bufs=8) as xtpool,
            tc.sbuf_pool(name="mid", bufs=5) as mpool,
            tc.sbuf_pool(name="rt", bufs=2) as rtpool,
            tc.sbuf_pool(name="sm", bufs=4) as smpool,
            tc.psum_pool(name="pmm", bufs=2) as pmmp,
            tc.psum_pool(name="pbc", bufs=3) as pbcp,
            tc.psum_pool(name="psc", bufs=3) as pscp,
        ):
            # ---- constants: one DMA, staged through DVE so consumers depend
            # on the DVE semaphore ----
            cst0 = cpool.tile([128, CST_W], BF16)
            nc.sync.dma_start(out=cst0[:], in_=cst_d[:])
            cst = cpool.tile([128, CST_W], BF16)
            nc.vector.tensor_copy(cst[:], cst0[:])
            ssel8 = cst[:, S_SSEL8:S_SSEL8 + 8]

            def sbc8(t):
                return cst[32 * t:32 * t + 8, S_SBC:S_SBC + 128]

            def csel(t):
                return cst[64 * t:64 * t + 40, S_CSEL:S_CSEL + 8]

            def cbc(t):
                return cst[32 * t:32 * t + 8, S_CBC:S_CBC + 40]

            jsel = cst[:80, S_JSEL:S_JSEL + 5]

            def jbc(t):
                return cst[32 * t:32 * t + 5, S_JBC:S_JBC + 80]
            wp_sb = cst[:, O_WP:O_WP + 768]
            wfm_sb = cst[:, O_WFM:O_WFM + 400]
            wf02_sb = cst[:, O_WF02:O_WF02 + 80]
            bp_sb = cst[:, O_BP:O_BP + 1]

            def bsel_c(c, t):
                return cst[64 * t:64 * t + 40,
                           S_BSEL + c * 128:S_BSEL + (c + 1) * 128]

            def ared_c(c):
                return cst[:, S_ARED + c * 40:S_ARED + (c + 1) * 40]

            def wv_c(c):
                return cst[:80, O_WV + c * 128:O_WV + (c + 1) * 128]

            class TS:
                """Per-tile state."""
                def __init__(self, it):
                    self.it = it

            class PairS:
                def __init__(self, k):
                    self.k = k
                    self.A = TS(2 * k)
                    self.B = TS(2 * k + 1)
                    self.ts = (self.A, self.B)

            def noop():
                pass

            # ---------------- phase H: load + mm1 + squash-u ----------------
            def h_dma(pr):
                for s in pr.ts:
                    s.xT = xtpool.tile([128, 6, NB], BF16, tag="xt")
                    src = x_d[:, s.it * NB:(s.it + 1) * NB].rearrange(
                        "(k p) b -> p k b", p=128)
                    nc.sync.dma_start(out=s.xT[:], in_=src)

            def h_mm1(pr, which):
                s = pr.ts[which]
                s.pu = pmmp.tile([128, NB], F32, tag="pmm")
                for k in range(6):
                    nc.tensor.matmul(
                        s.pu[:], wp_sb[:, k * 128:(k + 1) * 128],
                        s.xT[:, k, :], start=(k == 0), stop=(k == 5))

            def h_upre(pr, which):
                s = pr.ts[which]
                s.u_pre = mpool.tile([128, NB], BF16, tag="mid")
                nc.scalar.activation(s.u_pre[:], s.pu[:], AF.Identity,
                                     bias=bp_sb, scale=1.0)
                s.usq = mpool.tile([128, NB], BF16, tag="mid2")
                nc.scalar.activation(s.usq[:], s.pu[:], AF.Square,
                                     bias=bp_sb, scale=1.0)

            def h_psq(pr):
                # packed sq: tile A -> rows 0-7, tile B -> rows 32-39 of one
                # psum column; the gap rows hold garbage that is never read.
                psq = pbcp.tile([40, NB], F32, tag="pbc", name="psq")
                for t, s in enumerate(pr.ts):
                    nc.tensor.matmul(
                        psq[32 * t:32 * t + 8, :], ssel8,
                        s.usq[:], start=True, stop=True)
                w = smpool.tile([40, NB], F32, tag="sm")
                nc.scalar.activation(w[:], psq[:], AF.Ln)
                l1 = smpool.tile([40, NB], F32, tag="sm")
                nc.scalar.activation(l1[:], psq[:], AF.Ln,
                                     bias=1.0, scale=1.0)
                zf = smpool.tile([40, NB], F32, tag="sm")
                nc.vector.scalar_tensor_tensor(
                    zf[:], w[:], 0.5, l1[:],
                    op0=AluOpType.mult, op1=AluOpType.subtract)
                pr.fz = smpool.tile([40, NB], BF16, tag="smb")
                nc.scalar.activation(pr.fz[:], zf[:], AF.Exp)

            def h_u(pr):
                for t, s in enumerate(pr.ts):
                    pfb = pbcp.tile([128, NB], F32, tag="pbc")
                    nc.tensor.matmul(pfb[:], sbc8(t),
                                     pr.fz[32 * t:32 * t + 8, :],
                                     start=True, stop=True)
                    s.u = mpool.tile([128, NB], BF16, tag="mid3")
                    nc.vector.tensor_mul(s.u[:], s.u_pre[:], pfb[:])

            # ---------------- routing pieces ----------------
            def r_smm0(pr):
                for s in pr.ts:
                    s.psc = pscp.tile([80, NB], F32, tag="psc")
                    nc.tensor.matmul(s.psc[:], wf02_sb, s.u[:],
                                     start=True, stop=True)

            def r_soft(pr):
                # softmax on packed logits [104, NB] (tile B at row 64)
                pr.e = rtpool.tile([104, NB], BF16, tag="rt_e")
                nc.scalar.activation(pr.e[:], pr.logits[:], AF.Exp)
                pden = pbcp.tile([40, NB], F32, tag="pbc", name="pden")
                for t in range(2):
                    nc.tensor.matmul(pden[32 * t:32 * t + 8, :], csel(t),
                                     pr.e[64 * t:64 * t + 40, :],
                                     start=True, stop=True)
                rdr32 = smpool.tile([40, NB], F32, tag="sm")
                nc.vector.reciprocal_approx_fast(out=rdr32[:], in_=pden[:])
                rdr = smpool.tile([40, NB], BF16, tag="smb2")
                nc.scalar.copy(rdr[:], rdr32[:])
                pdb = pbcp.tile([104, NB], F32, tag="pbc", name="pdb")
                for t in range(2):
                    nc.tensor.matmul(pdb[64 * t:64 * t + 40, :], cbc(t),
                                     rdr[32 * t:32 * t + 8, :],
                                     start=True, stop=True)
                pr.cn = rtpool.tile([104, NB], BF16, tag="rt_cn")
                nc.vector.tensor_mul(pr.cn[:], pr.e[:], pdb[:])

            def r_s(prs):
                # per-class pipelined across the group:
                # [4x bcast mm] [4x ts mul] [4x wfm(c-1)]
                for pr in prs:
                    for s in pr.ts:
                        s.tsb = rtpool.tile([128, C, NB], BF16,
                                            tag="rt_ts", name="tsb",
                                            bufs=4)
                        s.pcb = [None] * C
                        s.psc = pscp.tile([80, NB], F32, tag="psc",
                                          name="psc")
                for c in range(C):
                    for pr in prs:
                        for t, s in enumerate(pr.ts):
                            pcb = pbcp.tile([128, NB], F32, tag="pbc",
                                            name="pcb")
                            nc.tensor.matmul(
                                pcb[:], bsel_c(c, t),
                                pr.cn[64 * t:64 * t + 40, :],
                                start=True, stop=True)
                            s.pcb[c] = pcb
                        for s in pr.ts:
                            nc.vector.tensor_mul(
                                s.tsb[:, c, :], s.u[:], s.pcb[c][:])
                    if c >= 1:
                        for pr in prs:
                            for s in pr.ts:
                                nc.tensor.matmul(
                                    s.psc[:],
                                    wfm_sb[:, (c - 1) * 80:c * 80],
                                    s.tsb[:, c - 1, :],
                                    start=(c == 1), stop=False)
                for pr in prs:
                    for s in pr.ts:
                        nc.tensor.matmul(
                            s.psc[:], wfm_sb[:, 4 * 80:5 * 80],
                            s.tsb[:, 4, :], start=False, stop=True)

            def r_v_head(pr):
                # s80 cast + ssq + jsel into packed vsq (rows 0-4 / 32-36)
                pvq = pbcp.tile([37, NB], F32, tag="pbc", name="pvq")
                pr.pvq = pvq
                for t, s in enumerate(pr.ts):
                    s.s80 = rtpool.tile([80, NB], BF16, tag=f"rt_s{t}",
                                        name="s80")
                    nc.scalar.copy(s.s80[:], s.psc[:])
                for t, s in enumerate(pr.ts):
                    s.ssq = rtpool.tile([80, NB], BF16, tag=f"rt_q{t}",
                                        name="ssq")
                    nc.scalar.activation(s.ssq[:], s.psc[:], AF.Square)
                for t, s in enumerate(pr.ts):
                    nc.tensor.matmul(pvq[32 * t:32 * t + 5, :],
                                     jsel, s.ssq[:], start=True, stop=True)

            def r_g(pr):
                pvq = pr.pvq
                wg = smpool.tile([37, NB], F32, tag="sm")
                nc.scalar.activation(wg[:], pvq[:], AF.Ln)
                lg1 = smpool.tile([37, NB], F32, tag="sm")
                nc.scalar.activation(lg1[:], pvq[:], AF.Ln,
                                     bias=1.0, scale=1.0)
                zg = smpool.tile([37, NB], F32, tag="sm")
                nc.vector.scalar_tensor_tensor(
                    zg[:], wg[:], 0.5, lg1[:],
                    op0=AluOpType.mult, op1=AluOpType.subtract)
                pr.g = smpool.tile([37, NB], BF16, tag="smb")
                nc.scalar.activation(pr.g[:], zg[:], AF.Exp)

            def r_v(pr, itr):
                for t, s in enumerate(pr.ts):
                    pgb = pbcp.tile([80, NB], F32, tag="pbc", name="pgb")
                    nc.tensor.matmul(pgb[:], jbc(t),
                                     pr.g[32 * t:32 * t + 5, :],
                                     start=True, stop=True)
                    dt = F32 if itr == 2 else BF16
                    s.v = rtpool.tile([80, NB], dt, tag=f"rt_v{t}", name="v")
                    nc.vector.tensor_mul(s.v[:], s.s80[:], pgb[:])
                    if itr == 2:
                        nc.sync.dma_start(
                            out=v_d[:, s.it * NB:(s.it + 1) * NB], in_=s.v[:])

            def r_ag(prs, itr):
                for pr in prs:
                    for s in pr.ts:
                        s.au = rtpool.tile([128, C, NB], BF16,
                                           tag="rt_au", name="au", bufs=4)
                        s.m = [None] * C
                    pr.pat = pscp.tile([104, NB], F32, tag="psc",
                                       name="pat")
                for c in range(C):
                    for pr in prs:
                        for t, s in enumerate(pr.ts):
                            m = pmmp.tile([128, NB], F32, tag="pmm",
                                          name="m")
                            nc.tensor.matmul(m[:], wv_c(c), s.v[:],
                                             start=True, stop=True)
                            s.m[c] = m
                        for s in pr.ts:
                            nc.vector.tensor_mul(
                                s.au[:, c, :], s.u[:], s.m[c][:])
                    if c >= 1:
                        for pr in prs:
                            for t, s in enumerate(pr.ts):
                                nc.tensor.matmul(
                                    pr.pat[64 * t:64 * t + 40, :],
                                    ared_c(c - 1), s.au[:, c - 1, :],
                                    start=(c == 1), stop=False)
                for pr in prs:
                    for t, s in enumerate(pr.ts):
                        nc.tensor.matmul(
                            pr.pat[64 * t:64 * t + 40, :], ared_c(4),
                            s.au[:, 4, :], start=False, stop=True)

            def r_logit(pr, itr):
                if itr == 0:
                    pr.logits = rtpool.tile([104, NB], F32, tag="rt_lg",
                                            name="logits")
                    nc.scalar.copy(pr.logits[:], pr.pat[:])
                else:
                    lg2 = rtpool.tile([104, NB], F32, tag="rt_lg2")
                    nc.vector.tensor_add(lg2[:], pr.logits[:], pr.pat[:])
                    pr.logits = lg2

            # ---------------- composed group phases ----------------
            def r0(prs):
                for pr in prs:
                    r_smm0(pr)
                for pr in prs:
                    r_v_head(pr)
                for pr in prs:
                    r_g(pr)
                for pr in prs:
                    r_v(pr, 0)
                r_ag(prs, 0)
                for pr in prs:
                    r_logit(pr, 0)

            def r12(prs, itr):
                for pr in prs:
                    r_soft(pr)
                r_s(prs)
                for pr in prs:
                    r_v_head(pr)
                for pr in prs:
                    r_g(pr)
                for pr in prs:
                    r_v(pr, itr)
                if itr == 1:
                    r_ag(prs, 1)
                    for pr in prs:
                        r_logit(pr, 1)

            pairs = [PairS(k) for k in range(npairs)]

            def h_all(prs):
                for pr in prs:
                    h_mm1(pr, 0)
                    h_mm1(pr, 1)
                for pr in prs:
                    h_upre(pr, 0)
                    h_upre(pr, 1)
                for pr in prs:
                    h_psq(pr)
                for pr in prs:
                    h_u(pr)

            assert npairs % 2 == 0
            groups = [pairs[2 * g:2 * g + 2] for g in range(npairs // 2)]
            for pr in groups[0]:
                h_dma(pr)
            for gi, grp in enumerate(groups):
                h_all(grp)
                if gi + 1 < len(groups):
                    for pr in groups[gi + 1]:
                        h_dma(pr)
                r0(grp)
                r12(grp, 1)
                r12(grp, 2)

    nc.compile()
    return nc


_NC_CACHE: dict = {}


def _get_nc(nt: int) -> bass.Bass:
    if nt not in _NC_CACHE:
        _NC_CACHE[nt] = build_nc(nt)
    return _NC_CACHE[nt]


def _prep_weights(Wp, bp, W):
    Wp = np.asarray(Wp, np.float32)
    bp = np.asarray(bp, np.float32)
    W = np.asarray(W, np.float32)
    wp_flat = Wp.transpose(1, 0, 2).reshape(768, 128)          # [d, (p,i)]
    wp_h = np.ascontiguousarray(
        wp_flat.reshape(6, 128, 128).transpose(1, 0, 2).reshape(128, 768))
    wflat_h = np.ascontiguousarray(
        W.transpose(0, 2, 1, 3).reshape(128, C * CD))          # [(p,i), (c,j)]
    wf02_h = np.ascontiguousarray(0.2 * wflat_h)
    wfm_h = np.zeros((128, C, 80), np.float32)
    for c in range(C):
        wfm_h[:, c, c * 16:(c + 1) * 16] = wflat_h[:, c * 16:(c + 1) * 16]
    wfm_h = np.ascontiguousarray(wfm_h.reshape(128, 400))
    # Wv_c[(c',j), (p,i)] = d_c'c W[p,c,i,j]  (class-masked, rhs is full v)
    wv_h = np.zeros((80, C, 128), np.float32)
    for c in range(C):
        wv_h[c * 16:(c + 1) * 16, c, :] = \
            W[:, c, :, :].transpose(2, 0, 1).reshape(16, 128)
    wv_h = np.ascontiguousarray(wv_h.reshape(80, C * 128))
    bp_h = np.ascontiguousarray(bp.reshape(128, 1))
    return wp_h, wflat_h, wf02_h, wfm_h, wv_h, bp_h


def pack_consts(Wp, bp, W):
    wp_h, wflat_h, wf02_h, wfm_h, wv_h, bp_h = _prep_weights(Wp, bp, W)
    cst = build_selectors()
    cst[:, O_WP:O_WP + 768] = wp_h
    cst[:, O_WFM:O_WFM + 400] = wfm_h
    cst[:, O_WF02:O_WF02 + 80] = wf02_h
    cst[:80, O_WV:O_WV + 640] = wv_h
    cst[:, O_BP:O_BP + 1] = bp_h
    return np.ascontiguousarray(cst.astype(ml_dtypes.bfloat16))


def make_in_maps(inputs, nt=NT):
    x = np.asarray(inputs["x"], np.float32)
    cst = pack_consts(inputs["Wp"], inputs["bp"], inputs["W"])
    bc = nt * NB
    return [{"xt": np.ascontiguousarray(
                x[i * BC:i * BC + bc].T.astype(ml_dtypes.bfloat16)),
             "cst": cst}
            for i in range(NCORES)]


def collect_out(res, nt=NT):
    # vout is [(c,j) = 80, bc] feature-major; transpose back to [b, c, j]
    outs = [np.asarray(res.results[i]["vout"]).reshape(C, CD, -1)
            .transpose(2, 0, 1) for i in range(NCORES)]
    return np.ascontiguousarray(np.concatenate(outs, axis=0).astype(np.float32))


def kernel(x, Wp, bp, W):
    nc = _get_nc(NT)
    in_maps = make_in_maps({"x": x, "Wp": Wp, "bp": bp, "W": W})
    res = run_bass_kernel_spmd(nc, in_maps, list(range(NCORES)))
    return collect_out(res)
